# revision 1
# baseline (speedup 1.0000x reference)
"""3-layer GCN (PyG GCNConv x3 + FC) on 8 Trainium2 NeuronCores.

Self-contained: host-side graph packing + Bass kernel + PJRT SPMD runner.

Sharding: core r owns destination nodes [r*12500, (r+1)*12500) and all their
incoming edges (plus self loops). Weights replicated. Per layer, each core
gathers source rows from a replicated feature table (AllGather between
layers), reduces per-destination via two matmul stages, applies bias +
leaky-relu, and pre-applies the next layer's weight matrix before the
AllGather.

All 8 cores execute ONE shared instruction stream; per-core variation is
entirely in streamed data:
  - gather index stream (int16, per-32768-row chunk of the table)
  - "segmap" per 128-edge block: [128 slots, 16] fp32 holding the GCN edge
    norm at (slot, block-local dst column)  -> stage-1 matmul
        pT[16 packed, 64 feat] = segmap.T @ gathered_msgs
    writes a disjoint 16-row stripe of a packed, transposed PSUM tile
  - "S" merge matrices [128 packed, 128 dst] of 0/1 -> stage-2 matmul
    accumulates packed columns into node-ordered window tiles [64, 512].
"""

import sys

sys.path.insert(0, "/opt/trn_rl_repo")

import numpy as np

N_NODES = 100000
N_EDGES = 3200000
IN_F, HID, N_CLS = 10, 64, 10
NEG_SLOPE = 0.01
N_CORES = 8
NS = N_NODES // N_CORES  # 12500 dst nodes per core
CHUNK = 32768  # table rows per gather chunk (int16 index range)
N_CHUNKS = (N_NODES + CHUNK - 1) // CHUNK  # 4
WIN = 512  # dst nodes per window (one PSUM bank of fp32)
N_WIN = (NS + WIN - 1) // WIN  # 25
BLK = 128  # edges per block (PE contraction dim)
STRIPE = 32  # dst columns representable per block
PT_BLKS = 4  # blocks per packed-transpose PSUM tile (32-row PE col groups)
CALL_BLKS = 8  # blocks per dma_gather call (1024 idx: SWDGE ring limit ~2K)
SUBW = 128  # dst per merge subwindow


# ---------------------------------------------------------------------------
# Host-side packing
# ---------------------------------------------------------------------------


def build_plan(edge_index):
    src = np.asarray(edge_index[0], dtype=np.int64)
    dst = np.asarray(edge_index[1], dtype=np.int64)
    deg = np.bincount(dst, minlength=N_NODES).astype(np.float64) + 1.0
    dinv = 1.0 / np.sqrt(deg)
    loop = np.arange(N_NODES, dtype=np.int64)
    s_all = np.concatenate([src, loop])
    d_all = np.concatenate([dst, loop])
    w_all = (dinv[s_all] * dinv[d_all]).astype(np.float32)

    # ---- per-core block packing -------------------------------------------
    # core -> window -> chunk -> list of blocks; each block: (slots' idx,
    # slots' col, slots' norm, block col->dst map)
    cores = []
    for r in range(N_CORES):
        sel = (d_all >= r * NS) & (d_all < (r + 1) * NS)
        es, ed, ew = s_all[sel], d_all[sel] - r * NS, w_all[sel]
        w_id = ed // WIN
        c_id = es // CHUNK
        order = np.lexsort((ed, c_id, w_id))
        es, ed, ew, w_id, c_id = (
            es[order],
            ed[order],
            ew[order],
            w_id[order],
            c_id[order],
        )
        key = w_id * N_CHUNKS + c_id
        run_starts = np.searchsorted(key, np.arange(N_WIN * N_CHUNKS), "left")
        run_ends = np.searchsorted(key, np.arange(N_WIN * N_CHUNKS), "right")

        # dense rank of dst within each run + first-occurrence index
        blocks = {}  # (w, c) -> list of (start, end, rank_at_start)
        n_e = len(es)
        newd = np.empty(n_e, dtype=bool)
        newd[0] = True
        newd[1:] = (ed[1:] != ed[:-1]) | (key[1:] != key[:-1])
        rank = np.cumsum(newd) - 1  # global rank, reset handled via diffs
        first_occ = np.flatnonzero(newd)  # index by rank
        for w in range(N_WIN):
            for c in range(N_CHUNKS):
                a, b = run_starts[w * N_CHUNKS + c], run_ends[w * N_CHUNKS + c]
                lst = []
                p = a
                while p < b:
                    r0 = rank[p]
                    # block may extend until 17th distinct dst or 128 edges
                    lim_rank = r0 + STRIPE
                    lim = first_occ[lim_rank] if lim_rank <= rank[b - 1] else b
                    q = min(p + BLK, lim, b)
                    lst.append((p, q, r0))
                    p = q
                blocks[(w, c)] = lst
        cores.append(
            dict(es=es, ed=ed, ew=ew, rank=rank, blocks=blocks)
        )

    # ---- global uniform structure -----------------------------------------
    b_max = np.zeros((N_WIN, N_CHUNKS), dtype=np.int64)
    for r in range(N_CORES):
        for w in range(N_WIN):
            for c in range(N_CHUNKS):
                b_max[w, c] = max(b_max[w, c], len(cores[r]["blocks"][(w, c)]))
    # pad each window's total block count to a multiple of PT_BLKS so the
    # packed-transpose PSUM tiles [128, 64] are always fully written
    for w in range(N_WIN):
        extra = (-b_max[w].sum()) % PT_BLKS
        b_max[w, N_CHUNKS - 1] += extra

    p0 = np.zeros((N_WIN, N_CHUNKS), dtype=np.int64)  # padded block prefix
    acc = 0
    pw0 = np.zeros(N_WIN + 1, dtype=np.int64)
    for w in range(N_WIN):
        pw0[w] = acc
        for c in range(N_CHUNKS):
            p0[w, c] = acc
            acc += b_max[w, c]
    pw0[N_WIN] = acc
    nblk_tot = acc
    pcw = (pw0[1:] - pw0[:-1]) * STRIPE  # packed cols per window

    # window sizes / merge geometry
    n_w = [min(WIN, NS - w * WIN) for w in range(N_WIN)]
    nslab = [(pcw[w] + 127) // 128 for w in range(N_WIN)]
    nsw = [(n_w[w] + SUBW - 1) // SUBW for w in range(N_WIN)]

    # ---- stream arrays -----------------------------------------------------
    t_idx = nblk_tot * BLK
    idx_streams = np.zeros((N_CORES, 16, t_idx // 16), dtype=np.int16)
    seg_streams = np.zeros((N_CORES, 128, nblk_tot * STRIPE), dtype=np.float32)
    pair_sets = [set() for _ in range(N_WIN)]
    pc2dst_all = []  # [core][w] -> array pcw[w]

    for r in range(N_CORES):
        co = cores[r]
        es, ew, ed, rank = co["es"], co["ew"], co["ed"], co["rank"]
        idx_flat = np.zeros(t_idx, dtype=np.int16)
        pc2dst_w = [np.full(pcw[w], -1, dtype=np.int64) for w in range(N_WIN)]
        for w in range(N_WIN):
            for c in range(N_CHUNKS):
                for j, (a, b, r0) in enumerate(co["blocks"][(w, c)]):
                    g = p0[w, c] + j
                    jw = g - pw0[w]
                    sl = slice(a, b)
                    slot = np.arange(b - a)
                    col = rank[sl] - r0
                    idx_flat[g * BLK + slot] = (es[sl] - c * CHUNK).astype(
                        np.int16
                    )
                    seg_streams[r][slot, g * STRIPE + col] = ew[sl]
                    # packed col -> window-local dst
                    pcs = jw * STRIPE + col
                    pc2dst_w[w][pcs] = ed[sl] - w * WIN
        idx_streams[r] = idx_flat.reshape(-1, 16).T
        pc2dst_all.append(pc2dst_w)
        for w in range(N_WIN):
            pc = np.flatnonzero(pc2dst_w[w] >= 0)
            if len(pc):
                for s, sw in set(
                    zip(pc // 128, pc2dst_w[w][pc] // SUBW)
                ):
                    pair_sets[w].add((int(s), int(sw)))

    # ordered pair list per window: by (sw, s) so each subwindow's
    # accumulation run is contiguous (clean start/stop flags)
    pairs = []  # per window: list of (s, sw, start, stop)
    pair_base = np.zeros(N_WIN + 1, dtype=np.int64)
    tot_pairs = 0
    for w in range(N_WIN):
        ordered = sorted(pair_sets[w], key=lambda t: (t[1], t[0]))
        lst = []
        for i, (s, sw) in enumerate(ordered):
            start = i == 0 or ordered[i - 1][1] != sw
            stop = i == len(ordered) - 1 or ordered[i + 1][1] != sw
            lst.append((s, sw, start, stop))
        pairs.append(lst)
        pair_base[w] = tot_pairs
        tot_pairs += len(lst)
    pair_base[N_WIN] = tot_pairs

    s_streams = np.zeros((N_CORES, 128, tot_pairs * SUBW), dtype=np.float32)
    for r in range(N_CORES):
        for w in range(N_WIN):
            p2d = pc2dst_all[r][w]
            for pi, (s, sw, _a, _b) in enumerate(pairs[w]):
                gp = pair_base[w] + pi
                i0 = s * 128
                rows = np.arange(i0, min(i0 + 128, pcw[w]))
                dloc = p2d[rows]
                m = (dloc >= 0) & (dloc // SUBW == sw)
                s_streams[r][rows[m] - i0, gp * SUBW + (dloc[m] % SUBW)] = 1.0

    idx_full = np.ascontiguousarray(
        np.tile(idx_streams, (1, 8, 1))
    )  # [cores, 128, t_idx//16]

    return dict(
        b_max=b_max,
        p0=p0,
        pw0=pw0,
        pcw=pcw,
        n_w=n_w,
        nslab=nslab,
        nsw=nsw,
        pairs=pairs,
        pair_base=pair_base,
        nblk_tot=int(nblk_tot),
        t_idx=int(t_idx),
        tot_pairs=int(tot_pairs),
        idx=idx_full,
        seg=seg_streams,
        smat=s_streams,
    )


# ---------------------------------------------------------------------------
# Kernel builder
# ---------------------------------------------------------------------------


def build_nc(plan, reps=1, sim_mode=False):
    import concourse.bacc as bacc
    from concourse import mybir
    from concourse.tile import TileContext

    dt = mybir.dt.float32
    b_max = plan["b_max"]
    p0 = plan["p0"]
    pw0 = plan["pw0"]
    pairs = plan["pairs"]
    pair_base = plan["pair_base"]
    n_w = plan["n_w"]

    nc = bacc.Bacc("TRN2", num_devices=1 if sim_mode else N_CORES)

    def allgather(t_loc, t_full):
        if sim_mode:
            # timing stand-in: copy the local shard into its full-table slice
            nc.sync.dma_start(out=t_full[0:NS, :], in_=t_loc[:])
        else:
            nc.gpsimd.collective_compute(
                "AllGather",
                mybir.AluOpType.bypass,
                replica_groups=[list(range(N_CORES))],
                ins=[t_loc[:]],
                outs=[t_full[:]],
            )

    x_pad = nc.dram_tensor("x_pad", [N_NODES, HID], dt, kind="ExternalInput")
    idx16 = nc.dram_tensor(
        "idx16", [128, plan["t_idx"] // 16], mybir.dt.int16, kind="ExternalInput"
    )
    seg = nc.dram_tensor(
        "seg", [128, plan["nblk_tot"] * STRIPE], dt, kind="ExternalInput"
    )
    smat = nc.dram_tensor(
        "smat", [128, plan["tot_pairs"] * SUBW], dt, kind="ExternalInput"
    )
    w1p = nc.dram_tensor("w1p", [HID, HID], dt, kind="ExternalInput")
    w2 = nc.dram_tensor("w2", [HID, HID], dt, kind="ExternalInput")
    w3 = nc.dram_tensor("w3", [HID, HID], dt, kind="ExternalInput")
    wfc = nc.dram_tensor("wfc", [HID, N_CLS], dt, kind="ExternalInput")
    b1 = nc.dram_tensor("b1", [HID, 1], dt, kind="ExternalInput")
    b2 = nc.dram_tensor("b2", [HID, 1], dt, kind="ExternalInput")
    b3 = nc.dram_tensor("b3", [HID, 1], dt, kind="ExternalInput")
    bfc = nc.dram_tensor("bfc", [128, N_CLS], dt, kind="ExternalInput")
    out = nc.dram_tensor("out", [NS, N_CLS], dt, kind="ExternalOutput")

    t2_loc = nc.dram_tensor("t2_loc", [NS, HID], dt)
    t3_loc = nc.dram_tensor("t3_loc", [NS, HID], dt)
    t2_full = nc.dram_tensor("t2_full", [N_NODES, HID], dt, addr_space="Shared")
    t3_full = nc.dram_tensor("t3_full", [N_NODES, HID], dt, addr_space="Shared")

    AF = mybir.ActivationFunctionType
    OP = mybir.AluOpType

    max_pt = max(
        int(-(-(pw0[w + 1] - pw0[w]) // PT_BLKS)) for w in range(N_WIN)
    )

    with TileContext(nc) as tc:
        with (
            tc.tile_pool(name="const", bufs=1) as cpool,
            tc.tile_pool(name="stream", bufs=2) as spool,
            tc.tile_pool(name="msg", bufs=4) as mpool,
            tc.tile_pool(name="pt", bufs=max_pt + 8) as ptpool,
            tc.tile_pool(name="hseg", bufs=2) as hpool,
            tc.tile_pool(name="rows", bufs=3) as rpool,
            tc.tile_pool(name="ppack", bufs=2, space="PSUM") as ppk,
            tc.tile_pool(name="pwin", bufs=2, space="PSUM") as pwn,
            tc.tile_pool(name="ptr", bufs=2, space="PSUM") as ptr,
        ):
            w1s = cpool.tile([HID, HID], dt, name="w1s")
            nc.sync.dma_start(out=w1s[:], in_=w1p[:])
            w2s = cpool.tile([HID, HID], dt, name="w2s")
            nc.sync.dma_start(out=w2s[:], in_=w2[:])
            w3s = cpool.tile([HID, HID], dt, name="w3s")
            nc.sync.dma_start(out=w3s[:], in_=w3[:])
            wfcs = cpool.tile([HID, N_CLS], dt, name="wfcs")
            nc.sync.dma_start(out=wfcs[:], in_=wfc[:])
            b1s = cpool.tile([HID, 1], dt, name="b1s")
            nc.sync.dma_start(out=b1s[:], in_=b1[:])
            b2s = cpool.tile([HID, 1], dt, name="b2s")
            nc.sync.dma_start(out=b2s[:], in_=b2[:])
            b3s = cpool.tile([HID, 1], dt, name="b3s")
            nc.sync.dma_start(out=b3s[:], in_=b3[:])
            bfcs = cpool.tile([128, N_CLS], dt, name="bfcs")
            nc.sync.dma_start(out=bfcs[:], in_=bfc[:])

            def layer(li, table, tnext_loc, bias_s, wnext_s):
                for w in range(N_WIN):
                    icol0 = int(pw0[w]) * (BLK // 16)
                    icols = int(b_max[w].sum()) * (BLK // 16)
                    idx_sl = spool.tile(
                        [128, icols], mybir.dt.int16, name="idx_sl", tag="idx"
                    )
                    nc.sync.dma_start(
                        out=idx_sl[:], in_=idx16[:, icol0 : icol0 + icols]
                    )
                    scol0 = int(pw0[w]) * STRIPE
                    scols = int(b_max[w].sum()) * STRIPE
                    seg_sl = spool.tile([128, scols], dt, name="seg_sl", tag="seg")
                    nc.sync.dma_start(
                        out=seg_sl[:], in_=seg[:, scol0 : scol0 + scols]
                    )
                    np_w = len(pairs[w])
                    sm_sl = spool.tile(
                        [128, np_w * SUBW], dt, name="sm_sl", tag="smat"
                    )
                    pcol0 = int(pair_base[w]) * SUBW
                    nc.sync.dma_start(
                        out=sm_sl[:], in_=smat[:, pcol0 : pcol0 + np_w * SUBW]
                    )

                    pt_sbufs = []
                    pt_psum = None
                    jw = 0
                    for c in range(N_CHUNKS):
                        bmax = int(b_max[w, c])
                        c0 = c * CHUNK
                        c1 = min(c0 + CHUNK, N_NODES)
                        done = 0
                        while done < bmax:
                            nblk = min(CALL_BLKS, bmax - done)
                            nidx = nblk * BLK
                            g0 = int(p0[w, c]) + done
                            coff = (g0 - int(pw0[w])) * (BLK // 16)
                            msg = mpool.tile(
                                [128, CALL_BLKS, HID], dt, name="msg", tag="msg"
                            )
                            nc.gpsimd.dma_gather(
                                out_ap=msg[:, :nblk, :],
                                in_ap=table[c0:c1, :],
                                idxs_ap=idx_sl[:, coff : coff + nidx // 16],
                                num_idxs=nidx,
                                num_idxs_reg=nidx,
                                elem_size=HID,
                            )
                            for jj in range(nblk):
                                if jw % PT_BLKS == 0:
                                    pt_psum = ppk.tile(
                                        [128, HID], dt, name="ptp", tag="ptp"
                                    )
                                prow = (jw % PT_BLKS) * STRIPE
                                nc.tensor.matmul(
                                    out=pt_psum[prow : prow + STRIPE, :],
                                    lhsT=seg_sl[
                                        :, jw * STRIPE : (jw + 1) * STRIPE
                                    ],
                                    rhs=msg[:, jj, :],
                                    start=True,
                                    stop=True,
                                    tile_position=(0, prow),
                                )
                                if jw % PT_BLKS == PT_BLKS - 1:
                                    pts = ptpool.tile(
                                        [128, HID], dt, name="pts", tag="pts"
                                    )
                                    nc.vector.tensor_copy(pts[:], pt_psum[:])
                                    pt_sbufs.append(pts)
                                jw += 1
                            done += nblk

                    win_ps = pwn.tile([HID, WIN], dt, name="win_ps", tag="win")
                    for pi, (s, sw, st, sp) in enumerate(pairs[w]):
                        nc.tensor.matmul(
                            out=win_ps[:, sw * SUBW : (sw + 1) * SUBW],
                            lhsT=pt_sbufs[s][:],
                            rhs=sm_sl[:, pi * SUBW : (pi + 1) * SUBW],
                            start=st,
                            stop=sp,
                        )

                    nw = n_w[w]
                    hT = hpool.tile([HID, WIN], dt, name="hT", tag="hT")
                    if li == 1:
                        agg_s = hpool.tile(
                            [HID, WIN], dt, name="agg_s", tag="agg"
                        )
                        nc.scalar.activation(
                            agg_s[:, :nw], win_ps[:, :nw], AF.Copy
                        )
                        h_ps = ptr.tile([HID, WIN], dt, name="h_ps", tag="hps")
                        nc.tensor.matmul(
                            out=h_ps[:, :nw],
                            lhsT=w1s[:],
                            rhs=agg_s[:, :nw],
                            start=True,
                            stop=True,
                        )
                        src_ps = h_ps
                    else:
                        src_ps = win_ps
                    # leaky_relu(x + b): t0 = x + b; hT = max(t0, 0.01*t0)
                    t0 = hpool.tile([HID, WIN], dt, name="t0", tag="t0")
                    nc.scalar.activation(
                        t0[:, :nw], src_ps[:, :nw], AF.Identity, bias=bias_s[:]
                    )
                    t1 = hpool.tile([HID, WIN], dt, name="t1", tag="t1")
                    nc.vector.tensor_scalar_mul(t1[:, :nw], t0[:, :nw], NEG_SLOPE)
                    nc.vector.tensor_tensor(
                        out=hT[:, :nw],
                        in0=t0[:, :nw],
                        in1=t1[:, :nw],
                        op=OP.max,
                    )

                    t0g = w * WIN
                    for tt in range(0, nw, 128):
                        tlen = min(128, nw - tt)
                        if li < 3:
                            tr = ptr.tile(
                                [128, HID], dt, name="tr", tag="tr"
                            )
                            nc.tensor.matmul(
                                out=tr[:tlen, :],
                                lhsT=hT[:, tt : tt + tlen],
                                rhs=wnext_s[:],
                                start=True,
                                stop=True,
                            )
                            rows = rpool.tile(
                                [128, HID], dt, name="rows", tag="rows"
                            )
                            nc.vector.tensor_copy(rows[:tlen, :], tr[:tlen, :])
                            nc.sync.dma_start(
                                out=tnext_loc[t0g + tt : t0g + tt + tlen, :],
                                in_=rows[:tlen, :],
                            )
                        else:
                            tr = ptr.tile(
                                [128, HID], dt, name="trf", tag="tr"
                            )
                            nc.tensor.matmul(
                                out=tr[:tlen, :N_CLS],
                                lhsT=hT[:, tt : tt + tlen],
                                rhs=wfcs[:],
                                start=True,
                                stop=True,
                            )
                            rows = rpool.tile(
                                [128, N_CLS], dt, name="rowsf", tag="rowsf"
                            )
                            nc.vector.tensor_tensor(
                                out=rows[:tlen, :],
                                in0=tr[:tlen, :N_CLS],
                                in1=bfcs[:tlen, :],
                                op=OP.add,
                            )
                            nc.sync.dma_start(
                                out=out[t0g + tt : t0g + tt + tlen, :],
                                in_=rows[:tlen, :],
                            )

            for _rep in range(reps):
                layer(1, x_pad, t2_loc, b1s, w2s)
                allgather(t2_loc, t2_full)
                layer(2, t2_full, t3_loc, b2s, w3s)
                allgather(t3_loc, t3_full)
                layer(3, t3_full, None, b3s, None)

    nc.finalize()
    return nc


# ---------------------------------------------------------------------------
# PJRT SPMD runner (build once, run many)
# ---------------------------------------------------------------------------


class _Runner:
    def __init__(self, nc, n_cores):
        import jax
        from jax.sharding import Mesh, PartitionSpec
        from jax.experimental.shard_map import shard_map
        from concourse import mybir
        from concourse.bass2jax import (
            _bass_exec_p,
            install_neuronx_cc_hook,
            partition_id_tensor,
        )

        install_neuronx_cc_hook()
        self.jax = jax
        self.n_cores = n_cores
        partition_name = (
            nc.partition_id_tensor.name if nc.partition_id_tensor else None
        )
        in_names, out_names, out_avals, zero_outs = [], [], [], []
        for alloc in nc.m.functions[0].allocations:
            if not isinstance(alloc, mybir.MemoryLocationSet):
                continue
            name = alloc.memorylocations[0].name
            if alloc.kind == "ExternalInput":
                if name != partition_name:
                    in_names.append(name)
            elif alloc.kind == "ExternalOutput":
                shape = tuple(alloc.tensor_shape)
                dtype = mybir.dt.np(alloc.dtype)
                out_names.append(name)
                out_avals.append(jax.core.ShapedArray(shape, dtype))
                zero_outs.append(np.zeros(shape, dtype))
        n_params = len(in_names)
        in_names = in_names + out_names
        if partition_name is not None:
            in_names.append(partition_name)
        self.in_names, self.n_params = in_names, n_params
        self.out_names, self.out_avals = out_names, out_avals
        self.zero_outs = zero_outs

        def _body(*args):
            operands = list(args)
            if partition_name is not None:
                operands.append(partition_id_tensor())
            return tuple(
                _bass_exec_p.bind(
                    *operands,
                    out_avals=tuple(out_avals),
                    in_names=tuple(in_names),
                    out_names=tuple(out_names),
                    lowering_input_output_aliases=(),
                    sim_require_finite=True,
                    sim_require_nnan=True,
                    nc=nc,
                )
            )

        devices = jax.devices()[:n_cores]
        self.mesh = Mesh(np.asarray(devices), ("core",))
        self.devices = devices
        self.PartitionSpec = PartitionSpec
        n_outs = len(out_avals)
        self.sharded = jax.jit(
            shard_map(
                _body,
                mesh=self.mesh,
                in_specs=(PartitionSpec("core"),) * (n_params + n_outs),
                out_specs=(PartitionSpec("core"),) * n_outs,
                check_rep=False,
            ),
            donate_argnums=tuple(range(n_params, n_params + n_outs)),
            keep_unused=True,
        )

    def prepare(self, in_maps):
        from jax.sharding import NamedSharding

        jax = self.jax
        n = self.n_cores
        sh = NamedSharding(self.mesh, self.PartitionSpec("core"))
        put = []
        for name in self.in_names[: self.n_params]:
            x = np.concatenate(
                [np.asarray(m[name]) for m in in_maps], axis=0
            )
            shards = np.split(x, n, axis=0)
            bufs = [
                jax.device_put(s, d)
                for s, d in zip(shards, self.devices, strict=True)
            ]
            put.append(
                jax.make_array_from_single_device_arrays(x.shape, sh, bufs)
            )
        jax.block_until_ready(put)
        return put

    def run(self, concat_in):
        n = self.n_cores
        zeros = [
            np.zeros((n * z.shape[0], *z.shape[1:]), z.dtype)
            for z in self.zero_outs
        ]
        outs = self.sharded(*concat_in, *zeros)
        self.jax.block_until_ready(outs)
        return outs

    def results(self, outs):
        n = self.n_cores
        return [
            {
                name: np.asarray(outs[i]).reshape(n, *self.out_avals[i].shape)[
                    c
                ]
                for i, name in enumerate(self.out_names)
            }
            for c in range(n)
        ]


# ---------------------------------------------------------------------------
# Entry point
# ---------------------------------------------------------------------------


def make_in_maps(plan, x, W1, b1, W2, b2, W3, b3, Wfc, bfc):
    x_pad = np.zeros((N_NODES, HID), np.float32)
    x_pad[:, :IN_F] = np.asarray(x, np.float32)
    w1p = np.zeros((HID, HID), np.float32)
    w1p[:IN_F, :] = np.asarray(W1, np.float32)
    base = dict(
        x_pad=x_pad,
        w1p=w1p,
        w2=np.asarray(W2, np.float32),
        w3=np.asarray(W3, np.float32),
        wfc=np.asarray(Wfc, np.float32),
        b1=np.asarray(b1, np.float32).reshape(HID, 1),
        b2=np.asarray(b2, np.float32).reshape(HID, 1),
        b3=np.asarray(b3, np.float32).reshape(HID, 1),
        bfc=np.tile(np.asarray(bfc, np.float32).reshape(1, N_CLS), (128, 1)),
    )
    return [
        dict(
            base,
            idx16=plan["idx"][r],
            seg=plan["seg"][r],
            smat=plan["smat"][r],
        )
        for r in range(N_CORES)
    ]


_CACHE = {}


def get_runner(plan, reps=1):
    key = ("nc", reps)
    if key not in _CACHE:
        nc = build_nc(plan, reps=reps)
        _CACHE[key] = _Runner(nc, N_CORES)
    return _CACHE[key]


def kernel(x, edge_index, W1, b1, W2, b2, W3, b3, Wfc, bfc):
    plan = build_plan(edge_index)
    runner = get_runner(plan, reps=1)
    in_maps = make_in_maps(plan, x, W1, b1, W2, b2, W3, b3, Wfc, bfc)
    ci = runner.prepare(in_maps)
    res = runner.results(runner.run(ci))
    return np.concatenate([res[r]["out"] for r in range(N_CORES)], axis=0)



# revision 21
# speedup vs baseline: 1.0233x; 1.0233x over previous
"""3-layer GCN (PyG GCNConv x3 + FC) on 8 Trainium2 NeuronCores.

Self-contained: host-side graph packing + Bass kernel + PJRT SPMD runner.

Sharding: core r owns destination nodes [r*12500, (r+1)*12500) and all their
incoming edges (plus self loops). Weights replicated. Per layer, each core
gathers source rows from a replicated feature table (AllGather between
layers), reduces per-destination via two matmul stages, applies bias +
leaky-relu, and pre-applies the next layer's weight matrix before the
AllGather.

All 8 cores execute ONE shared instruction stream; per-core variation is
entirely in streamed data:
  - gather index stream (int16, per-32768-row chunk of the table)
  - "segmap" per 128-edge block: [128 slots, 16] fp32 holding the GCN edge
    norm at (slot, block-local dst column)  -> stage-1 matmul
        pT[16 packed, 64 feat] = segmap.T @ gathered_msgs
    writes a disjoint 16-row stripe of a packed, transposed PSUM tile
  - "S" merge matrices [128 packed, 128 dst] of 0/1 -> stage-2 matmul
    accumulates packed columns into node-ordered window tiles [64, 512].
"""

import sys

sys.path.insert(0, "/opt/trn_rl_repo")

import numpy as np

N_NODES = 100000
N_EDGES = 3200000
IN_F, HID, N_CLS = 10, 64, 10
NEG_SLOPE = 0.01
N_CORES = 8
NS = N_NODES // N_CORES  # 12500 dst nodes per core
CHUNK = 32768  # table rows per gather chunk (int16 index range)
N_CHUNKS = (N_NODES + CHUNK - 1) // CHUNK  # 4
WIN = 512  # dst nodes per window (one PSUM bank of fp32)
N_WIN = (NS + WIN - 1) // WIN  # 25
BLK = 128  # edges per block (PE contraction dim)
STRIPE = 32  # dst columns representable per block
PT_BLKS = 4  # blocks per packed-transpose PSUM tile (32-row PE col groups)
CALL_BLKS = 8  # blocks per dma_gather call (1024 idx: SWDGE ring limit ~2K)
SUBW = 128  # dst per merge subwindow


# ---------------------------------------------------------------------------
# Host-side packing
# ---------------------------------------------------------------------------


def build_plan(edge_index):
    src = np.asarray(edge_index[0], dtype=np.int64)
    dst = np.asarray(edge_index[1], dtype=np.int64)
    deg = np.bincount(dst, minlength=N_NODES).astype(np.float64) + 1.0
    dinv = 1.0 / np.sqrt(deg)
    loop = np.arange(N_NODES, dtype=np.int64)
    s_all = np.concatenate([src, loop])
    d_all = np.concatenate([dst, loop])
    w_all = (dinv[s_all] * dinv[d_all]).astype(np.float32)

    # ---- per-core block packing -------------------------------------------
    # core -> window -> chunk -> list of blocks; each block: (slots' idx,
    # slots' col, slots' norm, block col->dst map)
    cores = []
    for r in range(N_CORES):
        sel = (d_all >= r * NS) & (d_all < (r + 1) * NS)
        es, ed, ew = s_all[sel], d_all[sel] - r * NS, w_all[sel]
        w_id = ed // WIN
        c_id = es // CHUNK
        order = np.lexsort((ed, c_id, w_id))
        es, ed, ew, w_id, c_id = (
            es[order],
            ed[order],
            ew[order],
            w_id[order],
            c_id[order],
        )
        key = w_id * N_CHUNKS + c_id
        run_starts = np.searchsorted(key, np.arange(N_WIN * N_CHUNKS), "left")
        run_ends = np.searchsorted(key, np.arange(N_WIN * N_CHUNKS), "right")

        # dense rank of dst within each run + first-occurrence index
        blocks = {}  # (w, c) -> list of (start, end, rank_at_start)
        n_e = len(es)
        newd = np.empty(n_e, dtype=bool)
        newd[0] = True
        newd[1:] = (ed[1:] != ed[:-1]) | (key[1:] != key[:-1])
        rank = np.cumsum(newd) - 1  # global rank, reset handled via diffs
        first_occ = np.flatnonzero(newd)  # index by rank
        for w in range(N_WIN):
            for c in range(N_CHUNKS):
                a, b = run_starts[w * N_CHUNKS + c], run_ends[w * N_CHUNKS + c]
                lst = []
                p = a
                while p < b:
                    r0 = rank[p]
                    # block may extend until 17th distinct dst or 128 edges
                    lim_rank = r0 + STRIPE
                    lim = first_occ[lim_rank] if lim_rank <= rank[b - 1] else b
                    q = min(p + BLK, lim, b)
                    lst.append((p, q, r0))
                    p = q
                blocks[(w, c)] = lst
        cores.append(
            dict(es=es, ed=ed, ew=ew, rank=rank, blocks=blocks)
        )

    # ---- global uniform structure -----------------------------------------
    b_max = np.zeros((N_WIN, N_CHUNKS), dtype=np.int64)
    for r in range(N_CORES):
        for w in range(N_WIN):
            for c in range(N_CHUNKS):
                b_max[w, c] = max(b_max[w, c], len(cores[r]["blocks"][(w, c)]))
    # pad each window's total block count to a multiple of PT_BLKS so the
    # packed-transpose PSUM tiles [128, 64] are always fully written
    for w in range(N_WIN):
        extra = (-b_max[w].sum()) % PT_BLKS
        b_max[w, N_CHUNKS - 1] += extra

    p0 = np.zeros((N_WIN, N_CHUNKS), dtype=np.int64)  # padded block prefix
    acc = 0
    pw0 = np.zeros(N_WIN + 1, dtype=np.int64)
    for w in range(N_WIN):
        pw0[w] = acc
        for c in range(N_CHUNKS):
            p0[w, c] = acc
            acc += b_max[w, c]
    pw0[N_WIN] = acc
    nblk_tot = acc
    pcw = (pw0[1:] - pw0[:-1]) * STRIPE  # packed cols per window

    # window sizes / merge geometry
    n_w = [min(WIN, NS - w * WIN) for w in range(N_WIN)]
    nslab = [(pcw[w] + 127) // 128 for w in range(N_WIN)]
    nsw = [(n_w[w] + SUBW - 1) // SUBW for w in range(N_WIN)]

    # ---- stream arrays -----------------------------------------------------
    t_idx = nblk_tot * BLK
    idx_streams = np.zeros((N_CORES, 16, t_idx // 16), dtype=np.int16)
    seg_streams = np.zeros((N_CORES, 128, nblk_tot * STRIPE), dtype=np.float32)
    pair_sets = [set() for _ in range(N_WIN)]
    pc2dst_all = []  # [core][w] -> array pcw[w]

    for r in range(N_CORES):
        co = cores[r]
        es, ew, ed, rank = co["es"], co["ew"], co["ed"], co["rank"]
        idx_flat = np.zeros(t_idx, dtype=np.int16)
        pc2dst_w = [np.full(pcw[w], -1, dtype=np.int64) for w in range(N_WIN)]
        for w in range(N_WIN):
            for c in range(N_CHUNKS):
                for j, (a, b, r0) in enumerate(co["blocks"][(w, c)]):
                    g = p0[w, c] + j
                    jw = g - pw0[w]
                    sl = slice(a, b)
                    slot = np.arange(b - a)
                    col = rank[sl] - r0
                    idx_flat[g * BLK + slot] = (es[sl] - c * CHUNK).astype(
                        np.int16
                    )
                    seg_streams[r][slot, g * STRIPE + col] = ew[sl]
                    # packed col -> window-local dst
                    pcs = jw * STRIPE + col
                    pc2dst_w[w][pcs] = ed[sl] - w * WIN
        idx_streams[r] = idx_flat.reshape(-1, 16).T
        pc2dst_all.append(pc2dst_w)
        for w in range(N_WIN):
            pc = np.flatnonzero(pc2dst_w[w] >= 0)
            if len(pc):
                for s, sw in set(
                    zip(pc // 128, pc2dst_w[w][pc] // SUBW)
                ):
                    pair_sets[w].add((int(s), int(sw)))

    # ordered pair list per window: by (sw, s) so each subwindow's
    # accumulation run is contiguous (clean start/stop flags)
    pairs = []  # per window: list of (s, sw, start, stop)
    pair_base = np.zeros(N_WIN + 1, dtype=np.int64)
    tot_pairs = 0
    for w in range(N_WIN):
        ordered = sorted(pair_sets[w], key=lambda t: (t[1], t[0]))
        lst = []
        for i, (s, sw) in enumerate(ordered):
            start = i == 0 or ordered[i - 1][1] != sw
            stop = i == len(ordered) - 1 or ordered[i + 1][1] != sw
            lst.append((s, sw, start, stop))
        pairs.append(lst)
        pair_base[w] = tot_pairs
        tot_pairs += len(lst)
    pair_base[N_WIN] = tot_pairs

    s_streams = np.zeros((N_CORES, 128, tot_pairs * SUBW), dtype=np.float32)
    for r in range(N_CORES):
        for w in range(N_WIN):
            p2d = pc2dst_all[r][w]
            for pi, (s, sw, _a, _b) in enumerate(pairs[w]):
                gp = pair_base[w] + pi
                i0 = s * 128
                rows = np.arange(i0, min(i0 + 128, pcw[w]))
                dloc = p2d[rows]
                m = (dloc >= 0) & (dloc // SUBW == sw)
                s_streams[r][rows[m] - i0, gp * SUBW + (dloc[m] % SUBW)] = 1.0

    idx_full = np.ascontiguousarray(
        np.tile(idx_streams, (1, 8, 1))
    )  # [cores, 128, t_idx//16]

    import ml_dtypes

    seg_streams = seg_streams.astype(ml_dtypes.bfloat16)
    s_streams = s_streams.astype(ml_dtypes.bfloat16)

    return dict(
        b_max=b_max,
        p0=p0,
        pw0=pw0,
        pcw=pcw,
        n_w=n_w,
        nslab=nslab,
        nsw=nsw,
        pairs=pairs,
        pair_base=pair_base,
        nblk_tot=int(nblk_tot),
        t_idx=int(t_idx),
        tot_pairs=int(tot_pairs),
        idx=idx_full,
        seg=seg_streams,
        smat=s_streams,
    )


# ---------------------------------------------------------------------------
# Kernel builder
# ---------------------------------------------------------------------------


def build_nc(
    plan,
    reps=1,
    sim_mode=False,
    skip_collective=False,
    scratch=81920,
    msg_bufs=8,
):
    import concourse.bacc as bacc
    from concourse import mybir
    from concourse.tile import TileContext

    dt = mybir.dt.float32
    bt = mybir.dt.bfloat16
    b_max = plan["b_max"]
    p0 = plan["p0"]
    pw0 = plan["pw0"]
    pairs = plan["pairs"]
    pair_base = plan["pair_base"]
    n_w = plan["n_w"]

    nc = bacc.Bacc(
        "TRN2",
        num_devices=1 if sim_mode else N_CORES,
        dynamic_dma_scratch_size=scratch,
    )

    def allgather(t_loc, t_full):
        if sim_mode or skip_collective:
            # timing stand-in: copy the local shard into its full-table slice
            nc.sync.dma_start(out=t_full[0:NS, :], in_=t_loc[:])
        else:
            nc.gpsimd.collective_compute(
                "AllGather",
                mybir.AluOpType.bypass,
                replica_groups=[list(range(N_CORES))],
                ins=[t_loc[:]],
                outs=[t_full[:]],
            )

    x_pad = nc.dram_tensor("x_pad", [N_NODES, HID], dt, kind="ExternalInput")
    idx16 = nc.dram_tensor(
        "idx16", [128, plan["t_idx"] // 16], mybir.dt.int16, kind="ExternalInput"
    )
    seg = nc.dram_tensor(
        "seg", [128, plan["nblk_tot"] * STRIPE], bt, kind="ExternalInput"
    )
    smat = nc.dram_tensor(
        "smat", [128, plan["tot_pairs"] * SUBW], bt, kind="ExternalInput"
    )
    w1p = nc.dram_tensor("w1p", [HID, HID], bt, kind="ExternalInput")
    w2 = nc.dram_tensor("w2", [HID, HID], bt, kind="ExternalInput")
    w3 = nc.dram_tensor("w3", [HID, HID], bt, kind="ExternalInput")
    wfc = nc.dram_tensor("wfc", [HID, N_CLS], bt, kind="ExternalInput")
    b1 = nc.dram_tensor("b1", [HID, 1], dt, kind="ExternalInput")
    b2 = nc.dram_tensor("b2", [HID, 1], dt, kind="ExternalInput")
    b3 = nc.dram_tensor("b3", [HID, 1], dt, kind="ExternalInput")
    bfc = nc.dram_tensor("bfc", [128, N_CLS], dt, kind="ExternalInput")
    out = nc.dram_tensor("out", [NS, N_CLS], dt, kind="ExternalOutput")

    t2_loc = nc.dram_tensor("t2_loc", [NS, HID], dt)
    t3_loc = nc.dram_tensor("t3_loc", [NS, HID], dt)
    t2_full = nc.dram_tensor("t2_full", [N_NODES, HID], dt, addr_space="Shared")
    t3_full = nc.dram_tensor("t3_full", [N_NODES, HID], dt, addr_space="Shared")

    AF = mybir.ActivationFunctionType
    OP = mybir.AluOpType

    max_pt = max(
        int(-(-(pw0[w + 1] - pw0[w]) // PT_BLKS)) for w in range(N_WIN)
    )

    with TileContext(nc) as tc:
        with (
            tc.tile_pool(name="const", bufs=1) as cpool,
            tc.tile_pool(name="stream", bufs=2) as spool,
            tc.tile_pool(name="msg", bufs=msg_bufs) as mpool,
            tc.tile_pool(name="msgb", bufs=4) as mbpool,
            tc.tile_pool(name="pt", bufs=max_pt + 8) as ptpool,
            tc.tile_pool(name="hseg", bufs=2) as hpool,
            tc.tile_pool(name="rows", bufs=3) as rpool,
            tc.tile_pool(name="ppack", bufs=2, space="PSUM") as ppk,
            tc.tile_pool(name="pwin", bufs=2, space="PSUM") as pwn,
            tc.tile_pool(name="ptr", bufs=2, space="PSUM") as ptr,
        ):
            w1s = cpool.tile([HID, HID], bt, name="w1s")
            nc.sync.dma_start(out=w1s[:], in_=w1p[:])
            w2s = cpool.tile([HID, HID], bt, name="w2s")
            nc.sync.dma_start(out=w2s[:], in_=w2[:])
            w3s = cpool.tile([HID, HID], bt, name="w3s")
            nc.sync.dma_start(out=w3s[:], in_=w3[:])
            wfcs = cpool.tile([HID, N_CLS], bt, name="wfcs")
            nc.sync.dma_start(out=wfcs[:], in_=wfc[:])
            b1s = cpool.tile([HID, 1], dt, name="b1s")
            nc.sync.dma_start(out=b1s[:], in_=b1[:])
            b2s = cpool.tile([HID, 1], dt, name="b2s")
            nc.sync.dma_start(out=b2s[:], in_=b2[:])
            b3s = cpool.tile([HID, 1], dt, name="b3s")
            nc.sync.dma_start(out=b3s[:], in_=b3[:])
            bfcs = cpool.tile([128, N_CLS], dt, name="bfcs")
            nc.sync.dma_start(out=bfcs[:], in_=bfc[:])

            def layer(li, table, tnext_loc, bias_s, wnext_s):
                for w in range(N_WIN):
                    icol0 = int(pw0[w]) * (BLK // 16)
                    icols = int(b_max[w].sum()) * (BLK // 16)
                    idx_sl = spool.tile(
                        [128, icols], mybir.dt.int16, name="idx_sl", tag="idx"
                    )
                    nc.sync.dma_start(
                        out=idx_sl[:], in_=idx16[:, icol0 : icol0 + icols]
                    )
                    scol0 = int(pw0[w]) * STRIPE
                    scols = int(b_max[w].sum()) * STRIPE
                    seg_sl = spool.tile([128, scols], bt, name="seg_sl", tag="seg")
                    nc.sync.dma_start(
                        out=seg_sl[:], in_=seg[:, scol0 : scol0 + scols]
                    )
                    np_w = len(pairs[w])
                    sm_sl = spool.tile(
                        [128, np_w * SUBW], bt, name="sm_sl", tag="smat"
                    )
                    pcol0 = int(pair_base[w]) * SUBW
                    nc.sync.dma_start(
                        out=sm_sl[:], in_=smat[:, pcol0 : pcol0 + np_w * SUBW]
                    )

                    pt_sbufs = []
                    pt_psum = None
                    jw = 0
                    for c in range(N_CHUNKS):
                        bmax = int(b_max[w, c])
                        c0 = c * CHUNK
                        c1 = min(c0 + CHUNK, N_NODES)
                        done = 0
                        while done < bmax:
                            nblk = min(CALL_BLKS, bmax - done)
                            nidx = nblk * BLK
                            g0 = int(p0[w, c]) + done
                            coff = (g0 - int(pw0[w])) * (BLK // 16)
                            msg = mpool.tile(
                                [128, CALL_BLKS, HID], dt, name="msg", tag="msg"
                            )
                            nc.gpsimd.dma_gather(
                                out_ap=msg[:, :nblk, :],
                                in_ap=table[c0:c1, :],
                                idxs_ap=idx_sl[:, coff : coff + nidx // 16],
                                num_idxs=nidx,
                                num_idxs_reg=nidx,
                                elem_size=HID,
                            )
                            msgb = mbpool.tile(
                                [128, CALL_BLKS, HID], bt, name="msgb", tag="msgb"
                            )
                            nc.scalar.activation(
                                msgb[:, :nblk, :], msg[:, :nblk, :], AF.Copy
                            )
                            for jj in range(nblk):
                                if jw % PT_BLKS == 0:
                                    pt_psum = ppk.tile(
                                        [128, HID], dt, name="ptp", tag="ptp"
                                    )
                                prow = (jw % PT_BLKS) * STRIPE
                                nc.tensor.matmul(
                                    out=pt_psum[prow : prow + STRIPE, :],
                                    lhsT=seg_sl[
                                        :, jw * STRIPE : (jw + 1) * STRIPE
                                    ],
                                    rhs=msgb[:, jj, :],
                                    start=True,
                                    stop=True,
                                    tile_position=(0, prow),
                                )
                                if jw % PT_BLKS == PT_BLKS - 1:
                                    pts = ptpool.tile(
                                        [128, HID], bt, name="pts", tag="pts"
                                    )
                                    nc.vector.tensor_copy(pts[:], pt_psum[:])
                                    pt_sbufs.append(pts)
                                jw += 1
                            done += nblk

                    win_ps = pwn.tile([HID, WIN], dt, name="win_ps", tag="win")
                    for pi, (s, sw, st, sp) in enumerate(pairs[w]):
                        nc.tensor.matmul(
                            out=win_ps[:, sw * SUBW : (sw + 1) * SUBW],
                            lhsT=pt_sbufs[s][:],
                            rhs=sm_sl[:, pi * SUBW : (pi + 1) * SUBW],
                            start=st,
                            stop=sp,
                        )

                    nw = n_w[w]
                    hT = hpool.tile([HID, WIN], bt, name="hT", tag="hT")
                    if li == 1:
                        agg_s = hpool.tile(
                            [HID, WIN], bt, name="agg_s", tag="agg"
                        )
                        nc.scalar.activation(
                            agg_s[:, :nw], win_ps[:, :nw], AF.Copy
                        )
                        h_ps = ptr.tile([HID, WIN], dt, name="h_ps", tag="hps")
                        nc.tensor.matmul(
                            out=h_ps[:, :nw],
                            lhsT=w1s[:],
                            rhs=agg_s[:, :nw],
                            start=True,
                            stop=True,
                        )
                        src_ps = h_ps
                    else:
                        src_ps = win_ps
                    # leaky_relu(x + b): t0 = x + b; hT = max(t0, 0.01*t0)
                    t0 = hpool.tile([HID, WIN], dt, name="t0", tag="t0")
                    nc.scalar.activation(
                        t0[:, :nw], src_ps[:, :nw], AF.Identity, bias=bias_s[:]
                    )
                    t1 = hpool.tile([HID, WIN], dt, name="t1", tag="t1")
                    nc.vector.tensor_scalar_mul(t1[:, :nw], t0[:, :nw], NEG_SLOPE)
                    nc.vector.tensor_tensor(
                        out=hT[:, :nw],
                        in0=t0[:, :nw],
                        in1=t1[:, :nw],
                        op=OP.max,
                    )

                    t0g = w * WIN
                    for tt in range(0, nw, 128):
                        tlen = min(128, nw - tt)
                        if li < 3:
                            tr = ptr.tile(
                                [128, HID], dt, name="tr", tag="tr"
                            )
                            nc.tensor.matmul(
                                out=tr[:tlen, :],
                                lhsT=hT[:, tt : tt + tlen],
                                rhs=wnext_s[:],
                                start=True,
                                stop=True,
                            )
                            rows = rpool.tile(
                                [128, HID], dt, name="rows", tag="rows"
                            )
                            nc.vector.tensor_copy(rows[:tlen, :], tr[:tlen, :])
                            nc.sync.dma_start(
                                out=tnext_loc[t0g + tt : t0g + tt + tlen, :],
                                in_=rows[:tlen, :],
                            )
                        else:
                            tr = ptr.tile(
                                [128, HID], dt, name="trf", tag="tr"
                            )
                            nc.tensor.matmul(
                                out=tr[:tlen, :N_CLS],
                                lhsT=hT[:, tt : tt + tlen],
                                rhs=wfcs[:],
                                start=True,
                                stop=True,
                            )
                            rows = rpool.tile(
                                [128, N_CLS], dt, name="rowsf", tag="rowsf"
                            )
                            nc.vector.tensor_tensor(
                                out=rows[:tlen, :],
                                in0=tr[:tlen, :N_CLS],
                                in1=bfcs[:tlen, :],
                                op=OP.add,
                            )
                            nc.sync.dma_start(
                                out=out[t0g + tt : t0g + tt + tlen, :],
                                in_=rows[:tlen, :],
                            )

            for _rep in range(reps):
                layer(1, x_pad, t2_loc, b1s, w2s)
                allgather(t2_loc, t2_full)
                layer(2, t2_full, t3_loc, b2s, w3s)
                allgather(t3_loc, t3_full)
                layer(3, t3_full, None, b3s, None)

    nc.finalize()
    return nc


# ---------------------------------------------------------------------------
# PJRT SPMD runner (build once, run many)
# ---------------------------------------------------------------------------


class _Runner:
    def __init__(self, nc, n_cores):
        import jax
        from jax.sharding import Mesh, PartitionSpec
        from jax.experimental.shard_map import shard_map
        from concourse import mybir
        from concourse.bass2jax import (
            _bass_exec_p,
            install_neuronx_cc_hook,
            partition_id_tensor,
        )

        install_neuronx_cc_hook()
        self.jax = jax
        self.n_cores = n_cores
        partition_name = (
            nc.partition_id_tensor.name if nc.partition_id_tensor else None
        )
        in_names, out_names, out_avals, zero_outs = [], [], [], []
        for alloc in nc.m.functions[0].allocations:
            if not isinstance(alloc, mybir.MemoryLocationSet):
                continue
            name = alloc.memorylocations[0].name
            if alloc.kind == "ExternalInput":
                if name != partition_name:
                    in_names.append(name)
            elif alloc.kind == "ExternalOutput":
                shape = tuple(alloc.tensor_shape)
                dtype = mybir.dt.np(alloc.dtype)
                out_names.append(name)
                out_avals.append(jax.core.ShapedArray(shape, dtype))
                zero_outs.append(np.zeros(shape, dtype))
        n_params = len(in_names)
        in_names = in_names + out_names
        if partition_name is not None:
            in_names.append(partition_name)
        self.in_names, self.n_params = in_names, n_params
        self.out_names, self.out_avals = out_names, out_avals
        self.zero_outs = zero_outs

        def _body(*args):
            operands = list(args)
            if partition_name is not None:
                operands.append(partition_id_tensor())
            return tuple(
                _bass_exec_p.bind(
                    *operands,
                    out_avals=tuple(out_avals),
                    in_names=tuple(in_names),
                    out_names=tuple(out_names),
                    lowering_input_output_aliases=(),
                    sim_require_finite=True,
                    sim_require_nnan=True,
                    nc=nc,
                )
            )

        devices = jax.devices()[:n_cores]
        self.mesh = Mesh(np.asarray(devices), ("core",))
        self.devices = devices
        self.PartitionSpec = PartitionSpec
        n_outs = len(out_avals)
        self.sharded = jax.jit(
            shard_map(
                _body,
                mesh=self.mesh,
                in_specs=(PartitionSpec("core"),) * (n_params + n_outs),
                out_specs=(PartitionSpec("core"),) * n_outs,
                check_rep=False,
            ),
            donate_argnums=tuple(range(n_params, n_params + n_outs)),
            keep_unused=True,
        )

    def prepare(self, in_maps):
        from jax.sharding import NamedSharding

        jax = self.jax
        n = self.n_cores
        sh = NamedSharding(self.mesh, self.PartitionSpec("core"))
        put = []
        for name in self.in_names[: self.n_params]:
            x = np.concatenate(
                [np.asarray(m[name]) for m in in_maps], axis=0
            )
            shards = np.split(x, n, axis=0)
            bufs = [
                jax.device_put(s, d)
                for s, d in zip(shards, self.devices, strict=True)
            ]
            put.append(
                jax.make_array_from_single_device_arrays(x.shape, sh, bufs)
            )
        jax.block_until_ready(put)
        return put

    def run(self, concat_in):
        n = self.n_cores
        zeros = [
            np.zeros((n * z.shape[0], *z.shape[1:]), z.dtype)
            for z in self.zero_outs
        ]
        outs = self.sharded(*concat_in, *zeros)
        self.jax.block_until_ready(outs)
        return outs

    def results(self, outs):
        n = self.n_cores
        return [
            {
                name: np.asarray(outs[i]).reshape(n, *self.out_avals[i].shape)[
                    c
                ]
                for i, name in enumerate(self.out_names)
            }
            for c in range(n)
        ]


# ---------------------------------------------------------------------------
# Entry point
# ---------------------------------------------------------------------------


def make_in_maps(plan, x, W1, b1, W2, b2, W3, b3, Wfc, bfc):
    import ml_dtypes

    bf16 = ml_dtypes.bfloat16
    x_pad = np.zeros((N_NODES, HID), np.float32)
    x_pad[:, :IN_F] = np.asarray(x, np.float32)
    w1p = np.zeros((HID, HID), np.float32)
    w1p[:IN_F, :] = np.asarray(W1, np.float32)
    base = dict(
        x_pad=x_pad,
        w1p=w1p.astype(bf16),
        w2=np.asarray(W2, np.float32).astype(bf16),
        w3=np.asarray(W3, np.float32).astype(bf16),
        wfc=np.asarray(Wfc, np.float32).astype(bf16),
        b1=np.asarray(b1, np.float32).reshape(HID, 1),
        b2=np.asarray(b2, np.float32).reshape(HID, 1),
        b3=np.asarray(b3, np.float32).reshape(HID, 1),
        bfc=np.tile(np.asarray(bfc, np.float32).reshape(1, N_CLS), (128, 1)),
    )
    return [
        dict(
            base,
            idx16=plan["idx"][r],
            seg=plan["seg"][r],
            smat=plan["smat"][r],
        )
        for r in range(N_CORES)
    ]


_CACHE = {}


def get_runner(plan, reps=1, **flags):
    key = ("nc", reps, tuple(sorted(flags.items())))
    if key not in _CACHE:
        nc = build_nc(plan, reps=reps, **flags)
        _CACHE[key] = _Runner(nc, N_CORES)
    return _CACHE[key]


def kernel(x, edge_index, W1, b1, W2, b2, W3, b3, Wfc, bfc):
    plan = build_plan(edge_index)
    runner = get_runner(plan, reps=1)
    in_maps = make_in_maps(plan, x, W1, b1, W2, b2, W3, b3, Wfc, bfc)
    ci = runner.prepare(in_maps)
    res = runner.results(runner.run(ci))
    return np.concatenate([res[r]["out"] for r in range(N_CORES)], axis=0)

